# revision 13
# baseline (speedup 1.0000x reference)
"""Trainium2 Bass kernel for fused attention + LayerNorm + projection.

Computation (per reference):
    q = input1 @ Wq + bq                       [8192, 32]
    k = input2 @ Wk + bk                       [8192, 32]
    v = input2 @ Wv (+ bv == 0)                [8192, 32]
    P = softmax(q @ k.T, axis=-1)              [8192, 8192]
    fused = P @ v                              [8192, 32]
    out = LayerNorm(fused) * gamma + beta @ Wo + bo   [8192, 128]

Sharding: data-parallel over rows of input1 (1024 rows per core, 8 cores);
input2 and weights replicated.

Algebraic simplifications (validated vs reference):
  - softmax normalization (and max-subtraction) skipped: LayerNorm is
    invariant to a positive per-row scale, so exp(s) @ v is normalized for
    free by LN.
  - v is never materialized: fused.T = Wv.T @ (x2.T @ P.T). The G = x2.T@P.T
    accumulation uses raw bf16 x2 chunks as stationary matmul weights,
    eliminating the per-chunk v matmuls and the banded AV reduce. This
    requires bv == 0 (true for this problem's inputs; host_inputs checks).
  - gamma folded into Wo (diag(gamma) @ Wo) and beta/bo folded into an
    extra contraction row via an augmented ones-row, on the host.

Dataflow per core (two m-passes of 512 q-rows):
  - all small constants arrive in ONE packed [128,580] DMA.
  - in2 chunks [128,128] PE-transposed (fp32) into a contiguous f32r
    i2t [128,512] per 4-chunk group; Pool also copies the raw chunks to a
    persistent bf16 x2b [128,8192].
  - kT per group: ONE wide f32r matmul [32,512] = Wk.T @ i2t, then Pool
    band-copies (+bk) into the banded kstack layout (chunk c at partitions
    32*(c%4)) used by the row-tiled score matmuls.
  - qT computed flat [32,1024] (2 wide f32r matmuls), replicated into all 4
    partition bands with a single stacked-identity broadcast matmul, +bq.
  - scoresT chunks via f32r row-tiled matmuls (K=32, 512-wide); exp on ACT
    straight out of PSUM into bf16 pt tiles. The G matmuls consuming pt are
    software-pipelined one step behind the score matmuls so the PE never
    waits on the ACT exp.
  - LayerNorm WITHOUT transposes: fusedT = Wv.T @ G lands in fsq[:, :512],
    DVE squares into fsq[:, 512:]; Sum(f)/Sum(f^2) via ones-vector matmuls;
    per-column affine (a,b) broadcast to 32 partitions by a selector matmul;
    normed+ones-row na (bf16) multiplies the folded Wo directly in [d, m].
  - pass-0 phase B runs mid-stream (hidden inside pass-1 attention) and
    computes rstd with a DVE fast-inverse-sqrt (bit trick + 1 Newton step) so
    the ACT exp table is never evicted mid-stream; pass-1 phase B at the tail
    uses ACT's exp(-0.5*ln(var)) instead (one table switch, shorter serial
    path).
"""

import os
import sys

import numpy as np

N1 = 8192
N2 = 8192
DIN = 128
D = 32
DOUT = 128
NCORES = 8
MSH = N1 // NCORES          # rows per core
NCH = N2 // 128             # 64 in2 chunks
NG = NCH // 4               # 16 groups of 4 chunks
LN_EPS = 1e-5
RSQRT_MAGIC = 0x5F3759DF

_CACHE = {}


def _import_concourse():
    try:
        import concourse.bass  # noqa: F401
    except ImportError:
        for p in ("/opt/trn_rl_repo", os.path.expanduser("~/.axon_site/_ro/trn_rl_repo")):
            if os.path.isdir(p) and p not in sys.path:
                sys.path.insert(0, p)


def build(reps=1):
    """Build (and cache) the compiled single-core SPMD Bass program."""
    key = ("nc", reps)
    if key in _CACHE:
        return _CACHE[key]
    _import_concourse()
    import concourse.bacc as bacc
    import concourse.tile as tile
    from concourse import mybir

    f32 = mybir.dt.float32
    i32 = mybir.dt.int32
    AF = mybir.ActivationFunctionType
    OP = mybir.AluOpType

    f32r = mybir.dt.float32r
    bf16 = mybir.dt.bfloat16

    nc = bacc.Bacc(None, target_bir_lowering=False, debug=False)

    x1 = nc.dram_tensor("x1", [MSH, DIN], f32, kind="ExternalInput")
    x2 = nc.dram_tensor("x2", [N2, DIN], f32, kind="ExternalInput")
    cpk_d = nc.dram_tensor("cpk", [128, 580], f32, kind="ExternalInput")
    out_d = nc.dram_tensor("out", [MSH, DOUT], f32, kind="ExternalOutput")

    from contextlib import ExitStack

    with tile.TileContext(nc) as tc, ExitStack() as outer:
        consts = outer.enter_context(tc.tile_pool(name="consts", bufs=1))
        persist = outer.enter_context(tc.tile_pool(name="persist", bufs=1))

        cpk = consts.tile([128, 580], f32)
        nc.sync.dma_start(out=cpk, in_=cpk_d[:])
        ident = cpk[:, 0:128]
        bq4 = cpk[:, 224:225]
        bk4 = cpk[:, 225:226]
        epsc = cpk[:, 226:227]

        wq_r = consts.tile([DIN, D], f32r)
        nc.vector.tensor_copy(wq_r, cpk[:, 128:160])
        wk_r = consts.tile([DIN, D], f32r)
        nc.vector.tensor_copy(wk_r, cpk[:, 160:192])
        wv_r = consts.tile([DIN, D], f32r)
        nc.vector.tensor_copy(wv_r, cpk[:, 192:224])
        woa_b = consts.tile([D + 1, DOUT], bf16)
        nc.vector.tensor_copy(woa_b, cpk[0:D + 1, 259:387])
        rep4_r = consts.tile([D, 128], f32r)
        nc.vector.tensor_copy(rep4_r, cpk[0:D, 387:515])
        ones_r = consts.tile([D, 1], f32r)
        nc.vector.tensor_copy(ones_r, cpk[0:D, 515:516])
        ones1_r = consts.tile([1, D], f32r)
        nc.vector.tensor_copy(ones1_r, cpk[0:1, 516:548])
        magic = consts.tile([1, 512], i32)
        nc.vector.memset(magic, RSQRT_MAGIC)

        # Pull the exp table load (~2.7us) into the initial DMA window.
        warm = consts.tile([1, 8], f32)
        nc.scalar.activation(warm, ident[0:1, 0:8], AF.Exp)

        kstack = persist.tile([128, NG * 128], f32r)    # kT chunk c: [32*(c%4):+32, 128*(c//4):+128]
        x2b = persist.tile([128, N2], bf16)             # raw in2 chunk c: [:, 128*c:+128]
        qt_rep = persist.tile([128, MSH], f32r)         # qT replicated in 4 bands
        na = persist.tile([D + 1, MSH], bf16)           # normed rows 0:32, ones row 32
        nc.vector.memset(na[D:D + 1, :], 1.0)

        for _rep in range(reps):
          with (
            tc.tile_pool(name="qload", bufs=1) as qload,
            tc.tile_pool(name="x2load", bufs=4) as x2load,
            tc.tile_pool(name="i2t_sb", bufs=2) as i2t_sbp,
            tc.tile_pool(name="sc_ps", bufs=2, space="PSUM") as sc_ps,
            tc.tile_pool(name="av_ps", bufs=1, space="PSUM") as av_ps,
            tc.tile_pool(name="pt", bufs=3) as ptp,
            tc.tile_pool(name="fsq", bufs=2) as fsqp,
            tc.tile_pool(name="gsb", bufs=2) as gsbp,
            tc.tile_pool(name="lnw", bufs=2) as lnwp,
            tc.tile_pool(name="outsb", bufs=2) as outsbp,
          ):
            # ---------------- helpers ----------------
            x2_tiles = {}

            def issue_x2_dma(g):
                t = x2load.tile([128, 4, 128], f32, tag="x2", name="x2sb")
                nc.sync.dma_start(
                    out=t,
                    in_=x2[g * 512:(g + 1) * 512, :].rearrange(
                        "(p t) d -> p t d", p=128
                    ),
                )
                x2_tiles[g] = t

            def prep_group(g, pp_ps):
                x2_sb = x2_tiles.pop(g)
                nc.gpsimd.tensor_copy(x2b[:, g * 512:(g + 1) * 512], x2_sb)
                i2t = i2t_sbp.tile([128, 512], f32r, tag="i2t")
                tps = pp_ps.tile([128, 512], f32, tag="pp", name="tps")
                for j in range(4):
                    nc.tensor.transpose(
                        tps[:, j * 128:(j + 1) * 128], x2_sb[:, j, :], ident
                    )
                nc.vector.tensor_copy(i2t, tps)
                ktps = pp_ps.tile([D, 512], f32, tag="pp", name="ktps")
                nc.tensor.matmul(ktps, lhsT=wk_r, rhs=i2t, start=True, stop=True)
                for j in range(4):
                    nc.vector.tensor_scalar_add(
                        kstack[32 * j:32 * (j + 1), g * 128:(g + 1) * 128],
                        ktps[:, j * 128:(j + 1) * 128],
                        bk4[32 * j:32 * (j + 1), :],
                    )

            x1_sb = None
            x1t = None
            qf_sb = None

            def issue_x1_dma():
                nonlocal x1_sb, x1t, qf_sb
                x1_sb = qload.tile([128, MSH // 128, 128], f32)
                for hh in range(2):
                    nc.sync.dma_start(
                        out=x1_sb[:, hh * 4:(hh + 1) * 4, :],
                        in_=x1[hh * 512:(hh + 1) * 512, :].rearrange(
                            "(t p) d -> p t d", p=128
                        ),
                    )
                x1t = qload.tile([128, MSH], f32r)
                qf_sb = qload.tile([D, MSH], f32r)

            def q_prep_half(h, pp_ps):
                tq = pp_ps.tile([128, 512], f32, tag="pp", name="tq")
                for tj in range(4):
                    t = h * 4 + tj
                    nc.tensor.transpose(
                        tq[:, tj * 128:(tj + 1) * 128], x1_sb[:, t, :], ident
                    )
                nc.vector.tensor_copy(x1t[:, h * 512:(h + 1) * 512], tq)
                qf_ps = pp_ps.tile([D, 512], f32, tag="pp", name="qf")
                nc.tensor.matmul(
                    qf_ps, lhsT=wq_r, rhs=x1t[:, h * 512:(h + 1) * 512],
                    start=True, stop=True,
                )
                nc.vector.tensor_copy(qf_sb[:, h * 512:(h + 1) * 512], qf_ps)
                qr_ps = pp_ps.tile([128, 512], f32, tag="pp", name="qr")
                nc.tensor.matmul(
                    qr_ps, lhsT=rep4_r, rhs=qf_sb[:, h * 512:(h + 1) * 512],
                    start=True, stop=True,
                )
                nc.vector.tensor_scalar_add(
                    qt_rep[:, h * 512:(h + 1) * 512], qr_ps, bq4
                )

            def attn_step(p, g, h, g_acc):
                """Emit scores+exp for (g,h); return deferred G-accum closure."""
                m0 = p * 512
                sps = sc_ps.tile([128, 1024], f32, tag="sc")
                for ci in range(2):
                    c = 4 * g + 2 * h + ci
                    j = c % 4
                    nc.tensor.matmul(
                        sps[:, 512 * ci:512 * (ci + 1)],
                        lhsT=kstack[32 * j:32 * (j + 1), g * 128:(g + 1) * 128],
                        rhs=qt_rep[32 * j:32 * (j + 1), m0:m0 + 512],
                        start=True,
                        stop=True,
                        tile_position=(32 * j, 0),
                    )
                pt = ptp.tile([128, 1024], bf16, tag="pt")
                nc.scalar.activation(pt, sps, AF.Exp)

                def emit_g():
                    for ci in range(2):
                        c = 4 * g + 2 * h + ci
                        nc.tensor.matmul(
                            g_acc,
                            lhsT=x2b[:, 128 * c:128 * (c + 1)],
                            rhs=pt[:, 512 * ci:512 * (ci + 1)],
                            start=(c == 0),
                            stop=(c == NCH - 1),
                            skip_group_check=True,
                        )
                return emit_g

            def fused_from_g(g_acc, fsq, phb_ps):
                g_sb = gsbp.tile([128, 512], f32r, tag="g")
                nc.vector.tensor_copy(g_sb, g_acc)
                f_ps = phb_ps.tile([D, 512], f32, tag="st", name="f_ps")
                nc.tensor.matmul(f_ps, lhsT=wv_r, rhs=g_sb, start=True, stop=True)
                nc.vector.tensor_copy(fsq[:, 0:512], f_ps)

            def ln_stats(fsq, phb_ps):
                """Common stats: returns (w, ve): mean in w[0:1], var in ve."""
                nc.vector.tensor_mul(fsq[:, 512:1024], fsq[:, 0:512], fsq[:, 0:512])
                s1 = phb_ps.tile([1, 512], f32, tag="st", name="s1")
                nc.tensor.matmul(s1, lhsT=ones_r, rhs=fsq[:, 0:512],
                                 start=True, stop=True)
                s2 = phb_ps.tile([1, 512], f32, tag="st", name="s2")
                nc.tensor.matmul(s2, lhsT=ones_r, rhs=fsq[:, 512:1024],
                                 start=True, stop=True)
                w = lnwp.tile([1, 1024], f32, tag="w")
                nc.vector.tensor_scalar_mul(w[:, 0:512], s1, 1.0 / D)   # mean
                nc.vector.tensor_mul(w[:, 512:1024], w[:, 0:512], w[:, 0:512])
                ve = lnwp.tile([1, 512], f32, tag="ve")
                nc.vector.scalar_tensor_tensor(
                    ve, s2, 1.0 / D, w[:, 512:1024], op0=OP.mult, op1=OP.subtract
                )
                return w, ve

            def phase_b_tail(p, fsq, phb_ps):
                """rstd via ACT ln/exp (one table switch; used at the tail)."""
                w, ve = ln_stats(fsq, phb_ps)
                lnv = lnwp.tile([1, 512], f32, tag="lnv")
                nc.scalar.activation(lnv, ve, AF.Ln, bias=epsc[0:1, :])
                a_t = lnwp.tile([1, 512], f32r, tag="ab")
                nc.scalar.activation(a_t, lnv, AF.Exp, scale=-0.5)
                finish_phase_b(p, fsq, phb_ps, w, a_t)

            def phase_b_mid(p, fsq, phb_ps):
                """rstd via DVE fast-inverse-sqrt (no ACT, exp table stays)."""
                w, ve = ln_stats(fsq, phb_ps)
                yi = lnwp.tile([1, 512], i32, tag="yi")
                nc.vector.tensor_scalar(
                    yi, ve.bitcast(i32), 1, None, op0=OP.logical_shift_right
                )
                y0 = lnwp.tile([1, 512], f32, tag="y0")
                nc.vector.tensor_sub(y0.bitcast(i32), magic, yi)
                hx = lnwp.tile([1, 512], f32, tag="hx")
                nc.gpsimd.tensor_scalar_mul(hx, ve, 0.5)
                yy = lnwp.tile([1, 512], f32, tag="yy")
                nc.vector.tensor_mul(yy, y0, y0)
                nc.gpsimd.tensor_mul(yy, yy, hx)            # h*y^2
                nc.vector.tensor_scalar(
                    yy, yy, -1.0, 1.5, op0=OP.mult, op1=OP.add
                )                                            # 1.5 - h*y^2
                a_t = lnwp.tile([1, 512], f32r, tag="ab")
                nc.vector.tensor_mul(a_t, y0, yy)            # a = rstd
                finish_phase_b(p, fsq, phb_ps, w, a_t)

            def finish_phase_b(p, fsq, phb_ps, w, a_t):
                m0 = p * 512
                # b = -mean * a
                b_t = lnwp.tile([1, 512], f32r, tag="bt")
                nc.vector.scalar_tensor_tensor(
                    b_t, w[:, 0:512], -1.0, a_t,
                    op0=OP.mult, op1=OP.mult,
                )
                bca = phb_ps.tile([D, 512], f32, tag="st", name="bca")
                nc.tensor.matmul(bca, lhsT=ones1_r, rhs=a_t, start=True, stop=True)
                bcb = phb_ps.tile([D, 512], f32, tag="st", name="bcb")
                nc.tensor.matmul(bcb, lhsT=ones1_r, rhs=b_t, start=True, stop=True)
                tmpn = lnwp.tile([D, 512], f32, tag="tmpn")
                nc.vector.tensor_mul(tmpn, fsq[:, 0:512], bca)
                nc.vector.tensor_add(na[0:D, m0:m0 + 512], tmpn, bcb)
                for mb in range(4):
                    pj = phb_ps.tile([128, DOUT], f32, tag="st", name="pj")
                    nc.tensor.matmul(
                        pj, lhsT=na[:, m0 + mb * 128:m0 + (mb + 1) * 128],
                        rhs=woa_b, start=True, stop=True,
                    )
                    osb = outsbp.tile([128, DOUT], f32, tag="o")
                    nc.vector.tensor_copy(osb, pj)
                    nc.sync.dma_start(
                        out=out_d[m0 + mb * 128:m0 + (mb + 1) * 128, :], in_=osb
                    )

            # ---------------- schedule ----------------
            g_acc0 = av_ps.tile([128, 512], f32, tag="av", name="gacc0")
            fsq0 = fsqp.tile([D, 1024], f32r, tag="fsq")
            with tc.tile_pool(name="pp_ps", bufs=2, space="PSUM") as pp_ps:
                issue_x2_dma(0)
                issue_x1_dma()
                issue_x2_dma(1)
                issue_x2_dma(2)
                prep_group(0, pp_ps)
                q_prep_half(0, pp_ps)
                pend = None
                for g in range(NG):
                    for h in range(2):
                        nxt = attn_step(0, g, h, g_acc0)
                        if pend is not None:
                            pend()
                        pend = nxt
                        if g == 0 and h == 0:
                            q_prep_half(1, pp_ps)
                    if g + 3 < NG:
                        issue_x2_dma(g + 3)
                    if g + 1 < NG:
                        prep_group(g + 1, pp_ps)
                pend()

            with tc.tile_pool(name="phb_ps", bufs=2, space="PSUM") as phb_ps:
                fused_from_g(g_acc0, fsq0, phb_ps)
                phase_b_mid(0, fsq0, phb_ps)
                g_acc1 = av_ps.tile([128, 512], f32, tag="av", name="gacc1")
                pend = None
                for g in range(NG):
                    for h in range(2):
                        nxt = attn_step(1, g, h, g_acc1)
                        if pend is not None:
                            pend()
                        pend = nxt
                pend()
                fsq1 = fsqp.tile([D, 1024], f32r, tag="fsq")
                fused_from_g(g_acc1, fsq1, phb_ps)
                phase_b_tail(1, fsq1, phb_ps)

    nc.compile()
    _CACHE[key] = nc
    return nc


def host_inputs(input1, input2, Wq, bq, Wk, bk, Wv, bv, gamma, beta, Wo, bo):
    """Per-core input maps (host-side weight folding)."""
    f32 = np.float32
    input1 = np.ascontiguousarray(np.asarray(input1, f32))
    input2 = np.ascontiguousarray(np.asarray(input2, f32))
    assert not np.any(np.asarray(bv)), (
        "kernel assumes bv == 0 (v never materialized on device)"
    )
    woa = np.concatenate(
        [np.asarray(gamma, f32)[:, None] * np.asarray(Wo, f32),
         (np.asarray(beta, f32) @ np.asarray(Wo, f32) + np.asarray(bo, f32))[None, :]],
        axis=0,
    ).astype(f32)
    i32 = np.eye(D, dtype=f32)
    rep4 = np.concatenate([i32] * 4, axis=1)          # [32, 128]
    cpk = np.zeros((128, 580), f32)
    cpk[:, 0:128] = np.eye(128, dtype=f32)
    cpk[:, 128:160] = np.asarray(Wq, f32)
    cpk[:, 160:192] = np.asarray(Wk, f32)
    cpk[:, 192:224] = np.asarray(Wv, f32)
    cpk[:, 224] = np.tile(np.asarray(bq, f32), 4)
    cpk[:, 225] = np.tile(np.asarray(bk, f32), 4)
    cpk[:, 226] = LN_EPS
    cpk[0:D + 1, 259:387] = woa
    cpk[0:D, 387:515] = rep4
    cpk[0:D, 515] = 1.0
    cpk[0, 516:548] = 1.0
    common = {
        "x2": input2,
        "cpk": cpk,
    }
    return [
        dict(common, x1=input1[c * MSH:(c + 1) * MSH]) for c in range(NCORES)
    ]


def kernel(input1, input2, Wq, bq, Wk, bk, Wv, bv, gamma, beta, Wo, bo):
    _import_concourse()
    from concourse.bass_utils import run_bass_kernel_spmd

    nc = build()
    in_maps = host_inputs(
        input1, input2, Wq, bq, Wk, bk, Wv, bv, gamma, beta, Wo, bo
    )
    res = run_bass_kernel_spmd(nc, in_maps, list(range(NCORES)))
    return np.concatenate(
        [np.asarray(res.results[c]["out"]) for c in range(NCORES)], axis=0
    ).astype(np.float32)


# revision 14
# speedup vs baseline: 1084.7775x; 1084.7775x over previous
"""Trainium2 Bass kernel for fused attention + LayerNorm + projection.

Computation (per reference):
    q = input1 @ Wq + bq                       [8192, 32]
    k = input2 @ Wk + bk                       [8192, 32]
    v = input2 @ Wv (+ bv == 0)                [8192, 32]
    P = softmax(q @ k.T, axis=-1)              [8192, 8192]
    fused = P @ v                              [8192, 32]
    out = LayerNorm(fused) * gamma + beta @ Wo + bo   [8192, 128]

Sharding: data-parallel over rows of input1 (1024 rows per core, 8 cores);
input2 and weights replicated.

Algebraic simplifications (validated vs reference):
  - softmax normalization (and max-subtraction) skipped: LayerNorm is
    invariant to a positive per-row scale, so exp(s) @ v is normalized for
    free by LN.
  - v is never materialized: fused.T = Wv.T @ (x2.T @ P.T). The G = x2.T@P.T
    accumulation uses raw bf16 x2 chunks as stationary matmul weights,
    eliminating the per-chunk v matmuls and the banded AV reduce. This
    requires bv == 0 (true for this problem's inputs; host_inputs checks).
  - gamma folded into Wo (diag(gamma) @ Wo) and beta/bo folded into an
    extra contraction row via an augmented ones-row, on the host.

Dataflow per core (two m-passes of 512 q-rows):
  - all small constants arrive in ONE packed [128,580] DMA.
  - in2 chunks [128,128] PE-transposed (fp32) into a contiguous f32r
    i2t [128,512] per 4-chunk group; Pool also copies the raw chunks to a
    persistent bf16 x2b [128,8192].
  - kT per group: ONE wide f32r matmul [32,512] = Wk.T @ i2t, then Pool
    band-copies (+bk) into the banded kstack layout (chunk c at partitions
    32*(c%4)) used by the row-tiled score matmuls.
  - qT computed flat [32,1024] (2 wide f32r matmuls), replicated into all 4
    partition bands with a single stacked-identity broadcast matmul, +bq.
  - scoresT chunks via f32r row-tiled matmuls (K=32, 512-wide); exp on ACT
    straight out of PSUM into bf16 pt tiles. The G matmuls consuming pt are
    software-pipelined one step behind the score matmuls so the PE never
    waits on the ACT exp.
  - LayerNorm WITHOUT transposes: fusedT = Wv.T @ G lands in fsq[:, :512],
    DVE squares into fsq[:, 512:]; Sum(f)/Sum(f^2) via ones-vector matmuls;
    per-column affine (a,b) broadcast to 32 partitions by a selector matmul;
    normed+ones-row na (bf16) multiplies the folded Wo directly in [d, m].
  - pass-0 phase B runs mid-stream (hidden inside pass-1 attention) and
    computes rstd with a DVE fast-inverse-sqrt (bit trick + 1 Newton step) so
    the ACT exp table is never evicted mid-stream; pass-1 phase B at the tail
    uses ACT's exp(-0.5*ln(var)) instead (one table switch, shorter serial
    path).
"""

import os
import sys

import numpy as np

N1 = 8192
N2 = 8192
DIN = 128
D = 32
DOUT = 128
NCORES = 8
MSH = N1 // NCORES          # rows per core
NCH = N2 // 128             # 64 in2 chunks
NG = NCH // 4               # 16 groups of 4 chunks
LN_EPS = 1e-5
RSQRT_MAGIC = 0x5F3759DF

_CACHE = {}


def _import_concourse():
    try:
        import concourse.bass  # noqa: F401
    except ImportError:
        for p in ("/opt/trn_rl_repo", os.path.expanduser("~/.axon_site/_ro/trn_rl_repo")):
            if os.path.isdir(p) and p not in sys.path:
                sys.path.insert(0, p)


def build(reps=1):
    """Build (and cache) the compiled single-core SPMD Bass program."""
    key = ("nc", reps)
    if key in _CACHE:
        return _CACHE[key]
    _import_concourse()
    import concourse.bacc as bacc
    import concourse.tile as tile
    from concourse import mybir

    f32 = mybir.dt.float32
    i32 = mybir.dt.int32
    AF = mybir.ActivationFunctionType
    OP = mybir.AluOpType

    f32r = mybir.dt.float32r
    bf16 = mybir.dt.bfloat16

    nc = bacc.Bacc(None, target_bir_lowering=False, debug=False)

    x1 = nc.dram_tensor("x1", [MSH, DIN], f32, kind="ExternalInput")
    x2 = nc.dram_tensor("x2", [N2, DIN], f32, kind="ExternalInput")
    cpk_d = nc.dram_tensor("cpk", [128, 580], f32, kind="ExternalInput")
    out_d = nc.dram_tensor("out", [MSH, DOUT], f32, kind="ExternalOutput")

    from contextlib import ExitStack

    with tile.TileContext(nc) as tc, ExitStack() as outer:
        consts = outer.enter_context(tc.tile_pool(name="consts", bufs=1))
        persist = outer.enter_context(tc.tile_pool(name="persist", bufs=1))

        cpk = consts.tile([128, 580], f32)
        nc.sync.dma_start(out=cpk, in_=cpk_d[:])
        ident = cpk[:, 0:128]
        bq4 = cpk[:, 224:225]
        bk4 = cpk[:, 225:226]
        epsc = cpk[:, 226:227]

        wq_r = consts.tile([DIN, D], f32r)
        nc.vector.tensor_copy(wq_r, cpk[:, 128:160])
        wk_r = consts.tile([DIN, D], f32r)
        nc.vector.tensor_copy(wk_r, cpk[:, 160:192])
        wv_r = consts.tile([DIN, D], f32r)
        nc.vector.tensor_copy(wv_r, cpk[:, 192:224])
        woa_b = consts.tile([D + 1, DOUT], bf16)
        nc.vector.tensor_copy(woa_b, cpk[0:D + 1, 259:387])
        rep4_r = consts.tile([D, 128], f32r)
        nc.vector.tensor_copy(rep4_r, cpk[0:D, 387:515])
        ones_r = consts.tile([D, 1], f32r)
        nc.vector.tensor_copy(ones_r, cpk[0:D, 515:516])
        ones1_r = consts.tile([1, D], f32r)
        nc.vector.tensor_copy(ones1_r, cpk[0:1, 516:548])
        magic = consts.tile([1, 512], i32)
        nc.vector.memset(magic, RSQRT_MAGIC)

        # Pull the exp table load (~2.7us) into the initial DMA window.
        warm = consts.tile([1, 8], f32)
        nc.scalar.activation(warm, ident[0:1, 0:8], AF.Exp)

        kstack = persist.tile([128, NG * 128], f32r)    # kT chunk c: [32*(c%4):+32, 128*(c//4):+128]
        x2b = persist.tile([128, N2], bf16)             # raw in2 chunk c: [:, 128*c:+128]
        qt_rep = persist.tile([128, MSH], f32r)         # qT replicated in 4 bands
        na = persist.tile([D + 1, MSH], bf16)           # normed rows 0:32, ones row 32
        nc.vector.memset(na[D:D + 1, :], 1.0)

        for _rep in range(reps):
          with (
            tc.tile_pool(name="qload", bufs=1) as qload,
            tc.tile_pool(name="x2load", bufs=4) as x2load,
            tc.tile_pool(name="i2t_sb", bufs=2) as i2t_sbp,
            tc.tile_pool(name="sc_ps", bufs=2, space="PSUM") as sc_ps,
            tc.tile_pool(name="av_ps", bufs=1, space="PSUM") as av_ps,
            tc.tile_pool(name="pt", bufs=3) as ptp,
            tc.tile_pool(name="fsq", bufs=2) as fsqp,
            tc.tile_pool(name="gsb", bufs=2) as gsbp,
            tc.tile_pool(name="lnw", bufs=2) as lnwp,
            tc.tile_pool(name="outsb", bufs=2) as outsbp,
          ):
            # ---------------- helpers ----------------
            x2_tiles = {}

            def issue_x2_dma(g):
                t = x2load.tile([128, 4, 128], f32, tag="x2", name="x2sb")
                nc.sync.dma_start(
                    out=t,
                    in_=x2[g * 512:(g + 1) * 512, :].rearrange(
                        "(p t) d -> p t d", p=128
                    ),
                )
                x2_tiles[g] = t

            def prep_group(g, pp_ps):
                x2_sb = x2_tiles.pop(g)
                nc.gpsimd.tensor_copy(x2b[:, g * 512:(g + 1) * 512], x2_sb)
                i2t = i2t_sbp.tile([128, 512], f32r, tag="i2t")
                tps = pp_ps.tile([128, 512], f32, tag="pp", name="tps")
                for j in range(4):
                    nc.tensor.transpose(
                        tps[:, j * 128:(j + 1) * 128], x2_sb[:, j, :], ident
                    )
                nc.vector.tensor_copy(i2t, tps)
                ktps = pp_ps.tile([D, 512], f32, tag="pp", name="ktps")
                nc.tensor.matmul(ktps, lhsT=wk_r, rhs=i2t, start=True, stop=True)
                for j in range(4):
                    nc.vector.tensor_scalar_add(
                        kstack[32 * j:32 * (j + 1), g * 128:(g + 1) * 128],
                        ktps[:, j * 128:(j + 1) * 128],
                        bk4[32 * j:32 * (j + 1), :],
                    )

            x1_sb = None
            x1t = None
            qf_sb = None

            def issue_x1_dma():
                nonlocal x1_sb, x1t, qf_sb
                x1_sb = qload.tile([128, MSH // 128, 128], f32)
                for hh in range(2):
                    nc.sync.dma_start(
                        out=x1_sb[:, hh * 4:(hh + 1) * 4, :],
                        in_=x1[hh * 512:(hh + 1) * 512, :].rearrange(
                            "(t p) d -> p t d", p=128
                        ),
                    )
                x1t = qload.tile([128, MSH], f32r)
                qf_sb = qload.tile([D, MSH], f32r)

            def q_prep_half(h, pp_ps):
                tq = pp_ps.tile([128, 512], f32, tag="pp", name="tq")
                for tj in range(4):
                    t = h * 4 + tj
                    nc.tensor.transpose(
                        tq[:, tj * 128:(tj + 1) * 128], x1_sb[:, t, :], ident
                    )
                nc.vector.tensor_copy(x1t[:, h * 512:(h + 1) * 512], tq)
                qf_ps = pp_ps.tile([D, 512], f32, tag="pp", name="qf")
                nc.tensor.matmul(
                    qf_ps, lhsT=wq_r, rhs=x1t[:, h * 512:(h + 1) * 512],
                    start=True, stop=True,
                )
                nc.vector.tensor_copy(qf_sb[:, h * 512:(h + 1) * 512], qf_ps)
                qr_ps = pp_ps.tile([128, 512], f32, tag="pp", name="qr")
                nc.tensor.matmul(
                    qr_ps, lhsT=rep4_r, rhs=qf_sb[:, h * 512:(h + 1) * 512],
                    start=True, stop=True,
                )
                nc.vector.tensor_scalar_add(
                    qt_rep[:, h * 512:(h + 1) * 512], qr_ps, bq4
                )

            def attn_step(p, g, h, g_acc):
                """Emit scores+exp for (g,h); return deferred G-accum closure."""
                m0 = p * 512
                sps = sc_ps.tile([128, 1024], f32, tag="sc")
                for ci in range(2):
                    c = 4 * g + 2 * h + ci
                    j = c % 4
                    nc.tensor.matmul(
                        sps[:, 512 * ci:512 * (ci + 1)],
                        lhsT=kstack[32 * j:32 * (j + 1), g * 128:(g + 1) * 128],
                        rhs=qt_rep[32 * j:32 * (j + 1), m0:m0 + 512],
                        start=True,
                        stop=True,
                        tile_position=(32 * j, 0),
                    )
                pt = ptp.tile([128, 1024], bf16, tag="pt")
                nc.scalar.activation(pt, sps, AF.Exp)

                def emit_g():
                    for ci in range(2):
                        c = 4 * g + 2 * h + ci
                        nc.tensor.matmul(
                            g_acc,
                            lhsT=x2b[:, 128 * c:128 * (c + 1)],
                            rhs=pt[:, 512 * ci:512 * (ci + 1)],
                            start=(c == 0),
                            stop=(c == NCH - 1),
                            skip_group_check=True,
                        )
                return emit_g

            def fused_from_g(g_acc, fsq, phb_ps):
                g_sb = gsbp.tile([128, 512], f32r, tag="g")
                nc.vector.tensor_copy(g_sb, g_acc)
                f_ps = phb_ps.tile([D, 512], f32, tag="st", name="f_ps")
                nc.tensor.matmul(f_ps, lhsT=wv_r, rhs=g_sb, start=True, stop=True)
                nc.vector.tensor_copy(fsq[:, 0:512], f_ps)

            def ln_stats(fsq, phb_ps):
                """Common stats: returns (w, ve): mean in w[0:1], var in ve."""
                nc.vector.tensor_mul(fsq[:, 512:1024], fsq[:, 0:512], fsq[:, 0:512])
                s1 = phb_ps.tile([1, 512], f32, tag="st", name="s1")
                nc.tensor.matmul(s1, lhsT=ones_r, rhs=fsq[:, 0:512],
                                 start=True, stop=True)
                s2 = phb_ps.tile([1, 512], f32, tag="st", name="s2")
                nc.tensor.matmul(s2, lhsT=ones_r, rhs=fsq[:, 512:1024],
                                 start=True, stop=True)
                w = lnwp.tile([1, 1024], f32, tag="w")
                nc.vector.tensor_scalar_mul(w[:, 0:512], s1, 1.0 / D)   # mean
                nc.vector.tensor_mul(w[:, 512:1024], w[:, 0:512], w[:, 0:512])
                ve = lnwp.tile([1, 512], f32, tag="ve")
                nc.vector.scalar_tensor_tensor(
                    ve, s2, 1.0 / D, w[:, 512:1024], op0=OP.mult, op1=OP.subtract
                )
                return w, ve

            def phase_b_mid(p, fsq, phb_ps):
                """rstd via DVE fast-inverse-sqrt (no ACT, exp table stays)."""
                w, ve = ln_stats(fsq, phb_ps)
                yi = lnwp.tile([1, 512], i32, tag="yi")
                nc.vector.tensor_scalar(
                    yi, ve.bitcast(i32), 1, None, op0=OP.logical_shift_right
                )
                y0 = lnwp.tile([1, 512], f32, tag="y0")
                nc.vector.tensor_sub(y0.bitcast(i32), magic, yi)
                hx = lnwp.tile([1, 512], f32, tag="hx")
                nc.gpsimd.tensor_scalar_mul(hx, ve, 0.5)
                yy = lnwp.tile([1, 512], f32, tag="yy")
                nc.vector.tensor_mul(yy, y0, y0)
                nc.vector.scalar_tensor_tensor(
                    yy, yy, -1.0, hx, op0=OP.mult, op1=OP.mult
                )                                            # -h*y^2
                a_t = lnwp.tile([1, 512], f32r, tag="ab")
                nc.vector.scalar_tensor_tensor(
                    a_t, yy, 1.5, y0, op0=OP.add, op1=OP.mult
                )                                            # y*(1.5 - h*y^2)
                finish_phase_b(p, fsq, phb_ps, w, a_t)

            def finish_phase_b(p, fsq, phb_ps, w, a_t):
                m0 = p * 512
                # b = -mean * a
                b_t = lnwp.tile([1, 512], f32r, tag="bt")
                nc.vector.scalar_tensor_tensor(
                    b_t, w[:, 0:512], -1.0, a_t,
                    op0=OP.mult, op1=OP.mult,
                )
                bca = phb_ps.tile([D, 512], f32, tag="st", name="bca")
                nc.tensor.matmul(bca, lhsT=ones1_r, rhs=a_t, start=True, stop=True)
                bcb = phb_ps.tile([D, 512], f32, tag="st", name="bcb")
                nc.tensor.matmul(bcb, lhsT=ones1_r, rhs=b_t, start=True, stop=True)
                tmpn = lnwp.tile([D, 512], f32, tag="tmpn")
                nc.vector.tensor_mul(tmpn, fsq[:, 0:512], bca)
                nc.vector.tensor_add(na[0:D, m0:m0 + 512], tmpn, bcb)
                for mb in range(4):
                    pj = phb_ps.tile([128, DOUT], f32, tag="st", name="pj")
                    nc.tensor.matmul(
                        pj, lhsT=na[:, m0 + mb * 128:m0 + (mb + 1) * 128],
                        rhs=woa_b, start=True, stop=True,
                    )
                    osb = outsbp.tile([128, DOUT], f32, tag="o")
                    nc.vector.tensor_copy(osb, pj)
                    nc.sync.dma_start(
                        out=out_d[m0 + mb * 128:m0 + (mb + 1) * 128, :], in_=osb
                    )

            # ---------------- schedule ----------------
            g_acc0 = av_ps.tile([128, 512], f32, tag="av", name="gacc0")
            fsq0 = fsqp.tile([D, 1024], f32r, tag="fsq")
            with tc.tile_pool(name="pp_ps", bufs=2, space="PSUM") as pp_ps:
                wrm = pp_ps.tile([128, 512], f32, tag="pp", name="wrm")
                for wj in range(4):
                    nc.tensor.transpose(
                        wrm[:, wj * 128:(wj + 1) * 128], cpk[:, 0:128], ident
                    )
                issue_x2_dma(0)
                issue_x1_dma()
                issue_x2_dma(1)
                issue_x2_dma(2)
                prep_group(0, pp_ps)
                q_prep_half(0, pp_ps)
                pend = None
                for g in range(NG):
                    for h in range(2):
                        nxt = attn_step(0, g, h, g_acc0)
                        if pend is not None:
                            pend()
                        pend = nxt
                        if g == 0 and h == 0:
                            q_prep_half(1, pp_ps)
                    if g + 3 < NG:
                        issue_x2_dma(g + 3)
                    if g + 1 < NG:
                        prep_group(g + 1, pp_ps)
                pend()

            with tc.tile_pool(name="phb_ps", bufs=2, space="PSUM") as phb_ps:
                fused_from_g(g_acc0, fsq0, phb_ps)
                phase_b_mid(0, fsq0, phb_ps)
                g_acc1 = av_ps.tile([128, 512], f32, tag="av", name="gacc1")
                pend = None
                for g in range(NG):
                    for h in range(2):
                        nxt = attn_step(1, g, h, g_acc1)
                        if pend is not None:
                            pend()
                        pend = nxt
                pend()
                fsq1 = fsqp.tile([D, 1024], f32r, tag="fsq")
                fused_from_g(g_acc1, fsq1, phb_ps)
                phase_b_mid(1, fsq1, phb_ps)

    nc.compile()
    _CACHE[key] = nc
    return nc


def host_inputs(input1, input2, Wq, bq, Wk, bk, Wv, bv, gamma, beta, Wo, bo):
    """Per-core input maps (host-side weight folding)."""
    f32 = np.float32
    input1 = np.ascontiguousarray(np.asarray(input1, f32))
    input2 = np.ascontiguousarray(np.asarray(input2, f32))
    assert not np.any(np.asarray(bv)), (
        "kernel assumes bv == 0 (v never materialized on device)"
    )
    woa = np.concatenate(
        [np.asarray(gamma, f32)[:, None] * np.asarray(Wo, f32),
         (np.asarray(beta, f32) @ np.asarray(Wo, f32) + np.asarray(bo, f32))[None, :]],
        axis=0,
    ).astype(f32)
    i32 = np.eye(D, dtype=f32)
    rep4 = np.concatenate([i32] * 4, axis=1)          # [32, 128]
    cpk = np.zeros((128, 580), f32)
    cpk[:, 0:128] = np.eye(128, dtype=f32)
    cpk[:, 128:160] = np.asarray(Wq, f32)
    cpk[:, 160:192] = np.asarray(Wk, f32)
    cpk[:, 192:224] = np.asarray(Wv, f32)
    cpk[:, 224] = np.tile(np.asarray(bq, f32), 4)
    cpk[:, 225] = np.tile(np.asarray(bk, f32), 4)
    cpk[:, 226] = LN_EPS
    cpk[0:D + 1, 259:387] = woa
    cpk[0:D, 387:515] = rep4
    cpk[0:D, 515] = 1.0
    cpk[0, 516:548] = 1.0
    common = {
        "x2": input2,
        "cpk": cpk,
    }
    return [
        dict(common, x1=input1[c * MSH:(c + 1) * MSH]) for c in range(NCORES)
    ]


def kernel(input1, input2, Wq, bq, Wk, bk, Wv, bv, gamma, beta, Wo, bo):
    _import_concourse()
    from concourse.bass_utils import run_bass_kernel_spmd

    nc = build()
    in_maps = host_inputs(
        input1, input2, Wq, bq, Wk, bk, Wv, bv, gamma, beta, Wo, bo
    )
    res = run_bass_kernel_spmd(nc, in_maps, list(range(NCORES)))
    return np.concatenate(
        [np.asarray(res.results[c]["out"]) for c in range(NCORES)], axis=0
    ).astype(np.float32)
